# revision 23
# baseline (speedup 1.0000x reference)
"""Trainium2 Bass kernel for CausalTensionGraphLayer.

Math (reference factorization):
  a   = x @ w1[:D]              [T, H]   (H = D/2)
  c   = x @ w1[D:]              [T, H]
  vzb = x @ wv_w + wv_b         [T, D]
  hid_w  = silu(a[t] + c[t-w-1] + b1)          (c term is 0 when t-w-1 < 0)
  tau_w  = sigmoid(hid_w @ w2 + b2)
  msg[t] = sum_w tau_w[t] * vzb[t-w-1]         (vzb -> wv_b when t-w-1 < 0)
  y      = x @ merge_w[:D] + msg @ merge_w[D:] + merge_b
  out    = LayerNorm(y) * gamma + beta

Neighbor gathers are row shifts of x, so with zero rows prepended for the
out-of-range halo the same compute path reproduces the reference exactly
(zero x rows give c = 0 and vzb = wv_b).

Sharding: data-parallel over the B*T = 8192 token rows, 1024 own tokens per
core plus a 4-row halo (zeros at batch boundaries, neighbor rows otherwise).
No collectives. Host pre-casts x/weights to bf16 and pre-transposes x so the
device works feature-major (tokens on the free axis -> shifts are free-dim
offsets).
"""

from contextlib import ExitStack

import numpy as np
import ml_dtypes

import concourse.bass as bass
import concourse.bacc as bacc
import concourse.tile as tile
from concourse import mybir
from concourse.bass_utils import run_bass_kernel_spmd

BF16 = ml_dtypes.bfloat16

B, T, D = 2, 4096, 1024
H = D // 2
W = 4
EPS = 1e-5
NCORES = 8
NTOK = (B * T) // NCORES          # 1024 own tokens per core
HALO = W                          # 4
GRID = NTOK + HALO                # 1028 (halo + own)
NQ = 4                            # quarters per core (pipeline granularity)
QT = NTOK // NQ                   # 256 own tokens per quarter
QG = QT + HALO                    # 260: shifted-grid cols per quarter
KD = D // 128                     # 8 K-chunks over D
MH = H // 128                     # 4 M-tiles over H
MD = D // 128                     # 8 M-tiles over D

FP32 = mybir.dt.float32
I32 = mybir.dt.int32
BF = mybir.dt.bfloat16
AF = mybir.ActivationFunctionType
ALU = mybir.AluOpType
AX = mybir.AxisListType


def build_nc(use_gamma_beta: bool, use_merge_b: bool):
    nc = bacc.Bacc(None, target_bir_lowering=False)

    xT = nc.dram_tensor("xT", [D, GRID], BF, kind="ExternalInput")
    w1a = nc.dram_tensor("w1a", [D, H], BF, kind="ExternalInput")
    w1c = nc.dram_tensor("w1c", [D, H], BF, kind="ExternalInput")
    wv = nc.dram_tensor("wv", [D, D], BF, kind="ExternalInput")
    m1 = nc.dram_tensor("m1", [D, D], BF, kind="ExternalInput")
    m2 = nc.dram_tensor("m2", [D, D], BF, kind="ExternalInput")
    w2rep = nc.dram_tensor("w2rep", [H, 128], BF, kind="ExternalInput")
    b1r = nc.dram_tensor("b1r", [128, MH], FP32, kind="ExternalInput")
    wvbr = nc.dram_tensor("wvbr", [128, MD], FP32, kind="ExternalInput")
    b2r = nc.dram_tensor("b2r", [128, 1], FP32, kind="ExternalInput")
    if use_gamma_beta:
        gam = nc.dram_tensor("gam", [1, D], FP32, kind="ExternalInput")
        bet = nc.dram_tensor("bet", [1, D], FP32, kind="ExternalInput")
    if use_merge_b:
        mbt = nc.dram_tensor("mbt", [1, D], FP32, kind="ExternalInput")
    y = nc.dram_tensor("y", [NTOK, D], FP32, kind="ExternalOutput")

    with tile.TileContext(nc) as tc, ExitStack() as ctx:
        persist = ctx.enter_context(tc.tile_pool(name="persist", bufs=1))
        qpool = ctx.enter_context(tc.tile_pool(name="qpool", bufs=2))
        mpool = ctx.enter_context(tc.tile_pool(name="mpool", bufs=4))
        mpool2 = ctx.enter_context(tc.tile_pool(name="mpool2", bufs=2))
        opool = ctx.enter_context(tc.tile_pool(name="opool", bufs=3))
        ps_acc = ctx.enter_context(tc.tile_pool(name="ps_acc", bufs=4, space="PSUM"))
        ps_log = ctx.enter_context(tc.tile_pool(name="ps_log", bufs=1, space="PSUM"))
        ps_y = ctx.enter_context(tc.tile_pool(name="ps_y", bufs=3, space="PSUM"))

        # ---- persistent loads -------------------------------------------
        # Two HWDGE queues (sync + scalar), each loaded in need-order so the
        # PE never waits on a transfer queued behind a later-needed one.
        xT_sb = persist.tile([128, KD, GRID], BF, tag="xT")
        w1a_sb = persist.tile([128, KD, H], BF, tag="w1a")
        w1c_sb = persist.tile([128, KD, H], BF, tag="w1c")
        w2rep_sb = persist.tile([128, MH, 128], BF, tag="w2rep")
        wv_sb = persist.tile([128, KD, D], BF, tag="wv")
        m1_sb = persist.tile([128, KD, D], BF, tag="m1")
        m2_sb = persist.tile([128, KD, D], BF, tag="m2")
        xT_r = xT.rearrange("(n p) t -> p n t", p=128)
        w1a_r = w1a.rearrange("(n p) m -> p n m", p=128)
        w1c_r = w1c.rearrange("(n p) m -> p n m", p=128)
        wv_r = wv.rearrange("(n p) m -> p n m", p=128)
        m1_r = m1.rearrange("(n p) m -> p n m", p=128)
        m2_r = m2.rearrange("(n p) m -> p n m", p=128)
        Q1 = QT + HALO  # first quarter's grid
        # Need order: {xT q0, w1a} -> w1c -> wv -> {w2rep, biases, xT q1}
        # -> m1/m2 -> xT tail. Interleave across the two queues so each
        # arrives just before the PE consumes it.
        b1_sb = persist.tile([128, MH], FP32, tag="b1")
        wvb_sb = persist.tile([128, MD], FP32, tag="wvb")
        b2_sb = persist.tile([128, 1], FP32, tag="b2")
        for k in range(KD):  # ~0.5MB sync: xT q0
            nc.sync.dma_start(out=xT_sb[:, k, 0:Q1], in_=xT_r[:, k, 0:Q1])
        for k in range(KD):  # 1MB scalar: w1a
            nc.scalar.dma_start(out=w1a_sb[:, k, :], in_=w1a_r[:, k, :])
        for k in range(KD):  # 1MB sync: w1c
            nc.sync.dma_start(out=w1c_sb[:, k, :], in_=w1c_r[:, k, :])
        for k in range(KD):  # 2MB wv split across both queues
            eng = nc.scalar if k % 2 == 0 else nc.sync
            eng.dma_start(out=wv_sb[:, k, :], in_=wv_r[:, k, :])
        for k in range(MH):  # small: w2rep + biases on scalar
            nc.scalar.dma_start(
                out=w2rep_sb[:, k, :],
                in_=w2rep.rearrange("(n p) m -> p n m", p=128)[:, k, :],
            )
        nc.scalar.dma_start(out=b1_sb, in_=b1r[:, :])
        nc.scalar.dma_start(out=wvb_sb, in_=wvbr[:, :])
        nc.scalar.dma_start(out=b2_sb, in_=b2r[:, :])
        for k in range(KD):  # ~0.5MB sync: xT q1
            nc.sync.dma_start(
                out=xT_sb[:, k, Q1:Q1 + QT], in_=xT_r[:, k, Q1:Q1 + QT]
            )
        for k in range(KD):  # 4MB: m1 on sync, m2 on scalar
            nc.sync.dma_start(out=m1_sb[:, k, :], in_=m1_r[:, k, :])
            nc.scalar.dma_start(out=m2_sb[:, k, :], in_=m2_r[:, k, :])
        for k in range(KD):  # ~1MB: xT q2/q3 tail, split
            nc.sync.dma_start(
                out=xT_sb[:, k, Q1 + QT:Q1 + 2 * QT],
                in_=xT_r[:, k, Q1 + QT:Q1 + 2 * QT],
            )
            nc.scalar.dma_start(
                out=xT_sb[:, k, Q1 + 2 * QT:GRID],
                in_=xT_r[:, k, Q1 + 2 * QT:GRID],
            )
        magic_sb = persist.tile([128, 1], I32, tag="magic")
        nc.vector.memset(magic_sb, 0x5F3759DF)
        one_i = persist.tile([128, 1], I32, tag="onei")
        nc.vector.memset(one_i, 1)
        if use_gamma_beta:
            gam_sb = persist.tile([128, D], FP32, tag="gam")
            nc.sync.dma_start(out=gam_sb, in_=gam.partition_broadcast(128))
            bet_sb = persist.tile([128, D], FP32, tag="bet")
            nc.sync.dma_start(out=bet_sb, in_=bet.partition_broadcast(128))
        if use_merge_b:
            mb_sb = persist.tile([128, D], FP32, tag="mb")
            nc.sync.dma_start(out=mb_sb, in_=mbt.partition_broadcast(128))

        # ---- per-quarter pipeline ---------------------------------------
        for q in range(NQ):
            g0 = q * QT          # xT col of first halo token of this quarter
            # P1: a (own grid, N=QT, with b1 folded into the eviction),
            # c and vzb (shifted grid, N=QG)
            aq = qpool.tile([128, MH, QT], BF, tag="aq")
            for m in range(MH):
                ps = ps_acc.tile([128, QT], FP32, tag="acc")
                for k in range(KD):
                    nc.tensor.matmul(
                        ps,
                        w1a_sb[:, k, m * 128:(m + 1) * 128],
                        xT_sb[:, k, g0 + HALO:g0 + HALO + QT],
                        start=(k == 0),
                        stop=(k == KD - 1),
                    )
                nc.scalar.activation(
                    out=aq[:, m, :], in_=ps, func=AF.Identity,
                    bias=b1_sb[:, m:m + 1], scale=1.0,
                )
            cq = qpool.tile([128, MH, QG], BF, tag="cq")
            for m in range(MH):
                ps = ps_acc.tile([128, QG], FP32, tag="acc")
                for k in range(KD):
                    nc.tensor.matmul(
                        ps,
                        w1c_sb[:, k, m * 128:(m + 1) * 128],
                        xT_sb[:, k, g0:g0 + QG],
                        start=(k == 0),
                        stop=(k == KD - 1),
                    )
                nc.scalar.copy(out=cq[:, m, :], in_=ps)
            vzq = qpool.tile([128, MD, QG], BF, tag="vzq")
            for m in range(MD):
                ps = ps_acc.tile([128, QG], FP32, tag="acc")
                for k in range(KD):
                    nc.tensor.matmul(
                        ps,
                        wv_sb[:, k, m * 128:(m + 1) * 128],
                        xT_sb[:, k, g0:g0 + QG],
                        start=(k == 0),
                        stop=(k == KD - 1),
                    )
                nc.scalar.activation(
                    out=vzq[:, m, :], in_=ps, func=AF.Identity,
                    bias=wvb_sb[:, m:m + 1], scale=1.0,
                )
            # Odd-parity shifted copies (c1[j] = c[j+1], v1[j] = vzb[j+1]) so
            # every DVE read below lands on a 4B boundary — the 2B-dtype
            # 2x perf mode requires 4B alignment on all operands.
            c1 = qpool.tile([128, MH, QG], BF, tag="c1")
            nc.gpsimd.dma_start(out=c1[:, :, 0:QG - 1], in_=cq[:, :, 1:QG])
            v1 = qpool.tile([128, MD, QG], BF, tag="v1")
            nc.gpsimd.dma_start(out=v1[:, :, 0:QG - 1], in_=vzq[:, :, 1:QG])

            def c_shift(o):
                return cq[:, :, o:o + QT] if o % 2 == 0 else c1[:, :, o - 1:o - 1 + QT]

            def v_shift(o):
                return vzq[:, :, o:o + QT] if o % 2 == 0 else v1[:, :, o - 1:o - 1 + QT]

            # P2: hid_w = silu(z) = z * sigmoid(z), z = (a + b1) + shift(c, w+1).
            # Sigmoid keeps ScalarE in one activation-table set for the whole
            # kernel (silu/sqrt live in different sets; switching costs ~2.7us).
            # tau_w comes out of the matmul pre-broadcast across partitions
            # because w2 is replicated over all 128 PE columns. w's are paired
            # so the logits matmuls stream N=512.
            tauq = qpool.tile([128, W, QT], BF, tag="tauq")
            for p in range(W // 2):
                hs = mpool2.tile([128, MH, 2, QT], BF, tag="hs")
                for wi in range(2):
                    w = 2 * p + wi
                    nc.vector.tensor_add(
                        hs[:, :, wi, :], aq, c_shift(HALO - 1 - w)
                    )
                sg = mpool2.tile([128, MH, 2, QT], BF, tag="sg")
                nc.scalar.activation(out=sg, in_=hs, func=AF.Sigmoid)
                hss = mpool2.tile([128, MH, 2, QT], BF, tag="hids")
                nc.vector.tensor_mul(hss, hs, sg)
                pl = ps_log.tile([128, 2 * QT], FP32, tag="logit")
                for k in range(MH):
                    nc.tensor.matmul(
                        pl,
                        w2rep_sb[:, k, :],
                        hss[:, k, :, :],
                        start=(k == 0),
                        stop=(k == MH - 1),
                    )
                nc.scalar.activation(
                    out=tauq[:, 2 * p:2 * p + 2, :],
                    in_=pl.rearrange("p (a b) -> p a b", a=2),
                    func=AF.Sigmoid,
                    bias=b2_sb[:, 0:1], scale=1.0,
                )
            # P3: msg = sum_w tau_w * shift(vzb, w+1), bf16, fused 3D ops
            # (tau broadcast over the 8 d-tiles via a step-0 mid dimension).
            msgq = qpool.tile([128, MD, QT], BF, tag="msgq")

            def tau_b(w):
                s = tauq[:, w, :]
                return bass.AP(
                    tensor=s.tensor, offset=s.offset,
                    ap=[s.ap[0], [0, MD], s.ap[1]],
                )

            pw = []
            for w in range(W):
                pt = mpool.tile([128, MD, QT], BF, tag="pw")
                nc.vector.tensor_mul(pt, tau_b(w), v_shift(HALO - 1 - w))
                pw.append(pt)
                if w == 1:
                    m01 = mpool2.tile([128, MD, QT], BF, tag="m01")
                    nc.vector.tensor_add(m01, pw[0], pw[1])
            nc.vector.tensor_add(pw[3], pw[2], pw[3])
            nc.vector.tensor_add(msgq, m01, pw[3])
            # P4: y = x@m1 + msg@m2 (+mb). LN stats come from ScalarE
            # accum_out (Copy for sum, Square for sum-of-squares — both in
            # the sigmoid table set), which also evicts PSUM early.
            NT = QT // 128
            srow = mpool.tile([128, NT, 2], FP32, tag="srow")
            sqs = mpool.tile([128, NT, 2], FP32, tag="sqs")
            ysb = []
            for tt in range(NT):
                tok0 = g0 + 128 * tt  # own-token index of row 0 of this tile
                yt = opool.tile([128, D], FP32, tag="ysb")
                ysb.append(yt)
                for half in range(2):
                    n0 = half * 512
                    yps = ps_y.tile([128, 512], FP32, tag="y")
                    for k in range(KD):
                        nc.tensor.matmul(
                            yps,
                            xT_sb[:, k, HALO + tok0:HALO + tok0 + 128],
                            m1_sb[:, k, n0:n0 + 512],
                            start=(k == 0),
                            stop=False,
                        )
                    for k in range(KD):
                        nc.tensor.matmul(
                            yps,
                            msgq[:, k, 128 * tt:128 * tt + 128],
                            m2_sb[:, k, n0:n0 + 512],
                            start=False,
                            stop=(k == KD - 1),
                        )
                    if use_merge_b:
                        nc.vector.tensor_add(yps, yps, mb_sb[:, n0:n0 + 512])
                    nc.scalar.activation(
                        out=yt[:, n0:n0 + 512], in_=yps, func=AF.Copy,
                        accum_out=srow[:, tt, half:half + 1],
                    )
                    junk = mpool2.tile([128, 512], FP32, tag="junk")
                    nc.scalar.activation(
                        out=junk, in_=yps, func=AF.Square,
                        accum_out=sqs[:, tt, half:half + 1],
                    )
            # P5: LayerNorm finalize for the quarter's NT token tiles at once.
            # rstd via bit-trick + 2 Newton steps (keeps sqrt off ScalarE).
            ssum = mpool.tile([128, NT], FP32, tag="ssum")
            nc.vector.reduce_sum(out=ssum, in_=srow, axis=AX.X)
            qsum = mpool.tile([128, NT], FP32, tag="qsum")
            nc.vector.reduce_sum(out=qsum, in_=sqs, axis=AX.X)
            mean = mpool.tile([128, NT], FP32, tag="mean")
            nc.vector.tensor_scalar_mul(mean, ssum, 1.0 / D)
            m2e = mpool.tile([128, NT], FP32, tag="m2e")
            # mean^2 - eps
            nc.vector.scalar_tensor_tensor(
                out=m2e, in0=mean, scalar=1.0, in1=mean,
                op0=ALU.mult, op1=ALU.mult,
            )
            nc.vector.tensor_scalar_add(m2e, m2e, -EPS)
            veps = mpool.tile([128, NT], FP32, tag="veps")
            # q/D - (mean^2 - eps) = var + eps
            nc.vector.scalar_tensor_tensor(
                out=veps, in0=qsum, scalar=1.0 / D, in1=m2e,
                op0=ALU.mult, op1=ALU.subtract,
            )
            rbits = mpool.tile([128, NT], I32, tag="rbits")
            nc.vector.tensor_scalar(
                out=rbits, in0=veps.bitcast(I32), scalar1=one_i[:, 0:1],
                scalar2=None, op0=ALU.arith_shift_right,
            )
            nc.vector.tensor_tensor(
                out=rbits, in0=magic_sb.to_broadcast([128, NT]), in1=rbits,
                op=ALU.subtract,
            )
            rstd = rbits.bitcast(FP32)
            for _ in range(2):
                nt1 = mpool.tile([128, NT], FP32, tag="nt1")
                nc.vector.tensor_mul(nt1, rstd, rstd)
                nc.vector.tensor_mul(nt1, nt1, veps)
                nc.vector.tensor_scalar(
                    out=nt1, in0=nt1, scalar1=-0.5, scalar2=1.5,
                    op0=ALU.mult, op1=ALU.add,
                )
                nc.vector.tensor_mul(rstd, rstd, nt1)
            for tt in range(NT):
                tok0 = g0 + 128 * tt
                yo = opool.tile([128, D], FP32, tag="yo")
                nc.vector.tensor_scalar(
                    out=yo, in0=ysb[tt], scalar1=mean[:, tt:tt + 1],
                    scalar2=rstd[:, tt:tt + 1],
                    op0=ALU.subtract, op1=ALU.mult,
                )
                if use_gamma_beta:
                    nc.vector.tensor_mul(yo, yo, gam_sb)
                    nc.vector.tensor_add(yo, yo, bet_sb)
                nc.sync.dma_start(out=y[tok0:tok0 + 128, :], in_=yo)
    nc.compile()
    return nc


_CACHE: dict = {}


def _get_nc(use_gamma_beta: bool, use_merge_b: bool):
    key = (use_gamma_beta, use_merge_b)
    if key not in _CACHE:
        _CACHE[key] = build_nc(use_gamma_beta, use_merge_b)
    return _CACHE[key]


def kernel(x, w1, b1, w2, b2, wv_w, wv_b, merge_w, merge_b, gamma, beta):
    x = np.asarray(x, dtype=np.float32)
    w1 = np.asarray(w1, dtype=np.float32)
    b1 = np.asarray(b1, dtype=np.float32)
    w2 = np.asarray(w2, dtype=np.float32)
    b2 = np.asarray(b2, dtype=np.float32)
    wv_w = np.asarray(wv_w, dtype=np.float32)
    wv_b = np.asarray(wv_b, dtype=np.float32)
    merge_w = np.asarray(merge_w, dtype=np.float32)
    merge_b = np.asarray(merge_b, dtype=np.float32)
    gamma = np.asarray(gamma, dtype=np.float32)
    beta = np.asarray(beta, dtype=np.float32)

    use_gamma_beta = not (np.all(gamma == 1.0) and np.all(beta == 0.0))
    use_merge_b = bool(np.any(merge_b != 0.0))
    nc = _get_nc(use_gamma_beta, use_merge_b)

    x2 = x.reshape(B * T, D)
    shared = {
        "w1a": w1[:D].astype(BF16),
        "w1c": w1[D:].astype(BF16),
        "wv": wv_w.astype(BF16),
        "m1": merge_w[:D].astype(BF16),
        "m2": merge_w[D:].astype(BF16),
        "w2rep": np.ascontiguousarray(
            np.broadcast_to(w2.reshape(H, 1), (H, 128))
        ).astype(BF16),
        "b1r": np.ascontiguousarray(b1.reshape(MH, 128).T),
        "wvbr": np.ascontiguousarray(wv_b.reshape(MD, 128).T),
        "b2r": np.full((128, 1), float(b2[0]), np.float32),
    }
    if use_gamma_beta:
        shared["gam"] = gamma.reshape(1, D)
        shared["bet"] = beta.reshape(1, D)
    if use_merge_b:
        shared["mbt"] = merge_b.reshape(1, D)

    in_maps = []
    for c in range(NCORES):
        t0 = c * NTOK
        xs = np.zeros((GRID, D), np.float32)
        xs[HALO:] = x2[t0:t0 + NTOK]
        if t0 % T != 0:  # halo stays inside the same batch element
            xs[:HALO] = x2[t0 - HALO:t0]
        m = dict(shared)
        m["xT"] = np.ascontiguousarray(xs.T).astype(BF16)
        in_maps.append(m)

    res = run_bass_kernel_spmd(nc, in_maps, core_ids=list(range(NCORES)))
    out = np.concatenate([r["y"] for r in res.results], axis=0)
    return out.reshape(B, T, D).astype(np.float32)
